# revision 21
# baseline (speedup 1.0000x reference)
"""Trainium2 Bass kernel for CausalSelfAttention (q@q^T variant), 8-way
tensor-parallel over heads.

Reference semantics (B=2, T=2048, C=1024, H=16, Dh=64):
    qkv = x @ w_attn + b_attn ; q, k, v = split(qkv)      # k is UNUSED
    att = softmax(causal_mask(q @ q^T / sqrt(Dh)))         # note q@q^T (not k)
    y   = att @ v ; out = y @ w_proj + b_proj

Sharding: core c owns heads {2c, 2c+1}, both batches (tensor parallel).
After attention, an 8-way AllToAll (split in two head-halves so the first
overlaps remaining attention) redistributes y from head-sharded to
token-sharded; each core then projects (full feature dim) its own 512-row
slice of the flattened [B*T, C] output. b_proj added on host.

All matmuls run in float32r (tf32) at full PE rate.  Scores are computed
directly transposed, sT[key, query], valid because q@q^T is symmetric;
v gets a ones-column so att@v also yields the softmax denominator; causal
masking is an additive -1e30 on the PSUM scores before exp.  Emission
interleaves batch-0 attention with batch-1 qkv windows so PE/ACT work
hides the 16MB x DMA.
"""

import numpy as np

import concourse.bass as bass  # noqa: F401
import concourse.mybir as mybir
import concourse.tile as tile
from concourse import bacc
from concourse.bass_utils import run_bass_kernel_spmd
from concourse.masks import make_identity, make_upper_triangular

f32 = mybir.dt.float32
f32r = mybir.dt.float32r
bf16 = mybir.dt.bfloat16
Act = mybir.ActivationFunctionType

B, T, C, H, DH = 2, 2048, 1024, 16, 64
FT = B * T              # 4096 flat tokens
NCORES = 8
HL = 2                  # heads per core
FL = HL * DH            # 128 local features
NE = C // 128           # 8 contraction chunks
TW = 512                # qkv window
NW = FT // TW           # 8 windows
NTT = FT // 128         # 32 token tiles
TS = FT // NCORES       # 512 output rows per core
SCALE = 1.0 / 8.0       # 1/sqrt(DH)
VW = 66                 # v slot width: 64 v cols + ones col + even-pad
NEG = -1.0e30

_NC_CACHE = {}

# tuning knobs (read at build time; key the cache)
OPTS = {
    "psS_bufs": 2,
    "psO_bufs": 2,
    "eb_bufs": 4,
    "expg": 2,
    "split_a2a": True,
    "x_bf16": True,    # bf16 x/w path: halves x DMA; margin still ~5x vs gate
    "nrep": 1,         # timing only: emit the whole body N times in one NEFF
}


def build_nc(variant="spmd"):
    key = (variant, tuple(sorted(OPTS.items())))
    if key in _NC_CACHE:
        return _NC_CACHE[key]
    EXPG = OPTS["expg"]
    nc = bacc.Bacc(
        "TRN2",
        target_bir_lowering=False,
        debug=False,
        enable_asserts=True,
        num_devices=NCORES if variant == "spmd" else 1,
    )
    # f32r inputs are host-pre-rounded to tf32 bit patterns
    xdt = bf16 if OPTS.get("x_bf16", True) else f32r
    xT = nc.dram_tensor("xT", [C, FT], xdt, kind="ExternalInput")
    wq = nc.dram_tensor("wq", [C, FL], xdt, kind="ExternalInput")
    wv = nc.dram_tensor("wv", [C, FL], xdt, kind="ExternalInput")
    bq = nc.dram_tensor("bq", [FL, 1], f32, kind="ExternalInput")
    bv = nc.dram_tensor("bv", [FL, 1], f32, kind="ExternalInput")
    wp = nc.dram_tensor("wp", [C, C], bf16, kind="ExternalInput")
    out = nc.dram_tensor("out", [TS, C], f32, kind="ExternalOutput")

    with tile.TileContext(nc) as tc:
        with (
            tc.tile_pool(name="const", bufs=1) as const,
            tc.tile_pool(name="xt", bufs=2) as xpool,
            tc.tile_pool(name="vt", bufs=2) as vtpool,
            tc.tile_pool(name="eb", bufs=OPTS["eb_bufs"]) as epool,
            tc.tile_pool(name="nrm", bufs=3) as nrm,
            tc.tile_pool(name="dram", bufs=1, space="DRAM") as dpool,
        ):
            wq_sb = const.tile([128, NE, FL], xdt)
            wv_sb = const.tile([128, NE, FL], xdt)
            wp_sb = const.tile([128, NE, C], bf16)
            bq_sb = const.tile([FL, 1], f32)
            bv_sb = const.tile([FL, 1], f32)
            ident = const.tile([128, 128], f32)
            onz = const.tile([128, 2], bf16)   # [1.0, 0.0] per partition
            qT_sb = const.tile([128, FT], f32r)           # [f_local, b*T+t]
            v_sb = const.tile([128, NTT, HL * VW], bf16)  # [t_in_tile, tile, h*VW+(d|1|pad)]
            yT_sb = const.tile([64, HL, FT], bf16)        # [d, h, b*T+t]
            yTf_sb = const.tile([128, NE, TS], bf16)      # post-a2a [f, chunk, t]

            # small loads first; wp spread across windows below
            nc.sync.dma_start(out=wq_sb, in_=wq.ap().rearrange("(c p) f -> p c f", p=128))
            nc.sync.dma_start(out=wv_sb, in_=wv.ap().rearrange("(c p) f -> p c f", p=128))
            nc.sync.dma_start(out=bq_sb, in_=bq.ap())
            nc.sync.dma_start(out=bv_sb, in_=bv.ap())
            make_identity(nc, ident)
            nc.vector.memset(onz[:, 0:1], 1.0)
            nc.vector.memset(onz[:, 1:2], 0.0)
            # bf16 upper-triangular 0/1 mask for the diagonal 128x128 blocks
            triu1 = const.tile([128, 128], f32)
            make_upper_triangular(nc, triu1, val=1.0, diag=True)
            triu_bf = const.tile([128, 128], bf16)
            nc.vector.tensor_copy(triu_bf, triu1)
            # ones/pad columns of every v slot are static: write them once
            v4 = v_sb.rearrange("p n (h x) -> p n h x", x=VW)
            nc.vector.tensor_copy(
                v4[:, :, :, 64:66],
                onz.unsqueeze(1).unsqueeze(1).broadcast_to((128, NTT, HL, 2)),
            )

            _pools = {}
            xT_r = xT.ap().rearrange("(c p) t -> p c t", p=128)
            wp_r = wp.ap().rearrange("(c p) f -> p c f", p=128)

            def emit_window(w):
                """qkv for token window w: qT columns + v tiles."""
                xt = xpool.tile([128, NE, TW], xdt, tag="xt")
                nc.sync.dma_start(out=xt, in_=xT_r[:, :, w * TW:(w + 1) * TW])
                pq = _pools['psQV'].tile([128, TW], f32, tag="qv")
                for e in range(NE):
                    nc.tensor.matmul(
                        pq, lhsT=wq_sb[:, e, :], rhs=xt[:, e, :],
                        start=(e == 0), stop=(e == NE - 1),
                    )
                nc.vector.tensor_scalar_add(
                    qT_sb[:, w * TW:(w + 1) * TW], pq, bq_sb,
                )
                pv = _pools['psQV'].tile([128, TW], f32, tag="qv")
                for e in range(NE):
                    nc.tensor.matmul(
                        pv, lhsT=wv_sb[:, e, :], rhs=xt[:, e, :],
                        start=(e == 0), stop=(e == NE - 1),
                    )
                vt = vtpool.tile([128, TW], f32, tag="vt")
                nc.vector.tensor_scalar_add(vt, pv, bv_sb)
                for s in range(TW // 128):
                    tt = w * (TW // 128) + s
                    pt = _pools['psT'].tile([128, 128], f32, tag="pt")
                    nc.tensor.transpose(pt, vt[:, s * 128:(s + 1) * 128], ident)
                    dst = v_sb[:, tt, :].rearrange("p (h x) -> p h x", x=VW)
                    nc.vector.tensor_copy(
                        dst[:, :, 0:64], pt.rearrange("p (h d) -> p h d", d=DH),
                    )
                # spread the 4MB wp load across windows
                nc.sync.dma_start(out=wp_sb[:, w, :], in_=wp_r[:, w, :])

            # diagonal-block column trims (r = j - 4*qd): queries below the
            # block's first key row are fully masked, so scores/exp/av only
            # need a column suffix.  Score/av starts keep >=256 cols where it
            # is rate-free to do so (<256-col matmuls pay 4x cycles/row).
            TRIM_S = {0: 0, 1: 128, 2: 256, 3: 256}   # score matmul col start
            TRIM_E = {0: 0, 1: 128, 2: 256, 3: 384}   # exp/av col start

            def emit_quad(h, b, qd):
                """attention for head h, batch b, query quad qd (512 queries)."""
                po = h * 64
                nj = 4 * qd + 4
                oT = _pools['psO'].tile([VW, 512], f32, tag="oT")
                q0 = b * T + qd * 512
                rq = qT_sb[po:po + 64, q0:q0 + 512]
                for g0 in range(0, nj, EXPG):
                    gs = min(EXPG, nj - g0)
                    S = _pools['psS'].tile([128, EXPG * 512], f32, tag="S")
                    for k in range(gs):
                        j = g0 + k
                        r = j - 4 * qd
                        cs = TRIM_S[r] if r >= 0 else 0
                        nc.tensor.matmul(
                            S[:, k * 512 + cs:(k + 1) * 512],
                            lhsT=qT_sb[po:po + 64, b * T + j * 128: b * T + (j + 1) * 128],
                            rhs=rq[:, cs:512],
                            start=True, stop=True,
                        )
                    eb = epool.tile([128, EXPG * 512], bf16, tag="eb")
                    if g0 < 4 * qd:
                        # off-diagonal group: one full-width exp
                        nc.scalar.activation(
                            eb[:, 0:gs * 512], S[:, 0:gs * 512], Act.Exp, scale=SCALE,
                        )
                    for k in range(gs):
                        j = g0 + k
                        r = j - 4 * qd
                        ce = TRIM_E[r] if r >= 0 else 0
                        if r >= 0:
                            nc.scalar.activation(
                                eb[:, k * 512 + ce:(k + 1) * 512],
                                S[:, k * 512 + ce:(k + 1) * 512], Act.Exp, scale=SCALE,
                            )
                            # 0/1 triangular mask on the 128-col diagonal slab
                            rt = r * 128
                            nc.vector.tensor_mul(
                                eb[:, k * 512 + rt:k * 512 + rt + 128],
                                eb[:, k * 512 + rt:k * 512 + rt + 128], triu_bf,
                            )
                        nc.tensor.matmul(
                            oT[:, ce:512],
                            lhsT=v_sb[:, b * (T // 128) + j, h * VW:(h + 1) * VW],
                            rhs=eb[:, k * 512 + ce:(k + 1) * 512],
                            start=(j == 0), stop=(j == nj - 1 or r >= 0),
                            skip_group_check=True,
                        )
                rec = nrm.tile([1, 512], f32, tag="rec")
                nc.vector.reciprocal(rec, oT[64:65, :])
                recb = nrm.tile([64, 512], f32, tag="recb")
                nc.gpsimd.partition_broadcast(recb, rec)
                nc.vector.tensor_mul(
                    yT_sb[:, h, q0:q0 + 512], oT[0:64, :], recb,
                )
                # this quad is exactly peer (4b+qd)'s token slice: send now so
                # the collective launch isn't gated on 8 back-to-back copies
                # (gpsimd queue: keeps the SP queue free for x/receive DMAs)
                peer = 4 * b + qd
                if h == 0:
                    nc.gpsimd.dma_start(
                        out=a2a_bufs["h0"][0][peer], in_=yT_sb[:, h, q0:q0 + 512],
                    )
                else:
                    for s in range(2):
                        nc.gpsimd.dma_start(
                            out=a2a_bufs[f"h1t{s}"][0][peer],
                            in_=yT_sb[:, h, q0 + s * 256:q0 + (s + 1) * 256],
                        )

            # bounce buffers, split by head-half so a2a#1 overlaps B tail
            a2a_bufs = {}

            def emit_a2a(key, prows, tcols):
                """collective + receive for buffer `key`; lands in yTf
                partition rows `prows` / token cols `tcols`.  Head-half 1 is
                split into two token-half collectives that pipeline with each
                other and with the first projection tiles."""
                ain, aout = a2a_bufs[key]
                if variant == "spmd":
                    nc.gpsimd.collective_compute(
                        "AllToAll",
                        mybir.AluOpType.bypass,
                        replica_groups=[list(range(NCORES))],
                        ins=[ain.opt()],
                        outs=[aout.opt()],
                    )
                else:  # timeline-estimation stand-in
                    nc.sync.dma_start(out=aout[:], in_=ain[:])
                nc.sync.dma_start(
                    out=yTf_sb[prows[0]:prows[1], :, tcols[0]:tcols[1]],
                    in_=aout.rearrange("q d t -> d q t"),
                )

            # ---------------- emission schedule ----------------
            for rep in range(OPTS.get("nrep", 1)):
                for key, tw in (("h0", TS), ("h1t0", TS // 2), ("h1t1", TS // 2)):
                    a2a_bufs[key] = tuple(
                        dpool.tile([NCORES, 64, tw], bf16,
                                   name=f"a2a_{d}_{key}_r{rep}",
                                   tag=f"a2a_{d}_{key}_r{rep}")
                        for d in ("in", "out")
                    )
                with (
                    tc.tile_pool(name="psQV", bufs=1, space="PSUM") as psQV,
                    tc.tile_pool(name="psT", bufs=1, space="PSUM") as psT,
                    tc.tile_pool(name="psS", bufs=OPTS["psS_bufs"], space="PSUM") as psS,
                    tc.tile_pool(name="psO", bufs=OPTS["psO_bufs"], space="PSUM") as psO,
                ):
                    _pools.update(psQV=psQV, psT=psT, psS=psS, psO=psO)
                    NQ = OPTS.get('nq', T // 512)
                    for w in range(4):                  # batch-0 windows
                        emit_window(w)
                    bat0 = [(h, 0, qd) for qd in range(NQ) for h in range(HL)]
                    # interleave batch-1 windows among batch-0 attention quads
                    wleft = list(range(4, NW))
                    for i, (h, b, qd) in enumerate(bat0):
                        if i % 2 == 0 and wleft:
                            emit_window(wleft.pop(0))
                        emit_quad(h, b, qd)
                    for w in wleft:
                        emit_window(w)
                    for qd in range(NQ):                # batch-1 head-half 0
                        emit_quad(0, 1, qd)
                    if OPTS.get("do_c", True):
                        # hidden under the h1 attention tail
                        emit_a2a("h0", (0, 64), (0, TS))
                    for qd in range(NQ):                # batch-1 head-half 1
                        emit_quad(1, 1, qd)
                    if OPTS.get("do_c", True):
                        # exposed: two pipelined token-half collectives; the
                        # first projection tiles overlap the second transfer
                        emit_a2a("h1t0", (64, 128), (0, TS // 2))
                        emit_a2a("h1t1", (64, 128), (TS // 2, TS))

                # ---------------- output projection ----------------
                with (
                    tc.tile_pool(name="psP", bufs=4, space="PSUM") as psP,
                    tc.tile_pool(name="ob", bufs=4) as outpool,
                ):
                    for ttile in range(TS // 128 if OPTS.get("do_d", True) else 0):
                        for cc in range(C // 512):
                            pp = psP.tile([128, 512], f32, tag="pp")
                            for fc in range(NE):
                                nc.tensor.matmul(
                                    pp,
                                    lhsT=yTf_sb[:, fc, ttile * 128:(ttile + 1) * 128],
                                    rhs=wp_sb[:, fc, cc * 512:(cc + 1) * 512],
                                    start=(fc == 0), stop=(fc == NE - 1),
                                )
                            ob = outpool.tile([128, 512], f32, tag="ob")
                            nc.vector.tensor_copy(ob, pp)
                            # per-cc output DMA on the ACT queue (idle at tail)
                            nc.scalar.dma_start(
                                out=out.ap()[ttile * 128:(ttile + 1) * 128,
                                             cc * 512:(cc + 1) * 512],
                                in_=ob,
                            )

    nc.compile()
    _NC_CACHE[key] = nc
    return nc


def _round_tf32(a):
    u = np.ascontiguousarray(a, dtype=np.float32).view(np.uint32)
    r = ((u.astype(np.uint64) + 0x1000) & 0xFFFFE000).astype(np.uint32)
    return r.view(np.float32)


def make_in_maps(input_tokens, w_attn, b_attn, w_proj):
    x = np.ascontiguousarray(np.asarray(input_tokens, dtype=np.float32))
    w_attn = np.asarray(w_attn, dtype=np.float32)
    b_attn = np.asarray(b_attn, dtype=np.float32)
    w_proj = np.asarray(w_proj, dtype=np.float32)

    import ml_dtypes

    def bcast(a):
        return np.ascontiguousarray(a).astype(ml_dtypes.bfloat16)

    xcast = bcast if OPTS.get("x_bf16", True) else _round_tf32
    xT = xcast(np.ascontiguousarray(x.reshape(FT, C).T))  # [C, FT]
    wpr = bcast(np.ascontiguousarray(w_proj))
    in_maps = []
    for c in range(NCORES):
        f0 = c * FL
        in_maps.append({
            "xT": xT,
            "wq": xcast(np.ascontiguousarray(w_attn[:, f0:f0 + FL])),
            "wv": xcast(np.ascontiguousarray(w_attn[:, 2 * C + f0:2 * C + f0 + FL])),
            "bq": np.ascontiguousarray(b_attn[f0:f0 + FL].reshape(FL, 1)),
            "bv": np.ascontiguousarray(b_attn[2 * C + f0:2 * C + f0 + FL].reshape(FL, 1)),
            "wp": wpr,
        })
    return in_maps


def assemble(results, b_proj):
    flat = np.concatenate([results[c]["out"] for c in range(NCORES)], axis=0)
    flat = flat + np.asarray(b_proj, dtype=np.float32)[None, :]
    return flat.reshape(B, T, C)


def kernel(input_tokens, w_attn, b_attn, w_proj, b_proj, _stats=None):
    nc = build_nc()
    in_maps = make_in_maps(input_tokens, w_attn, b_attn, w_proj)
    trace = _stats is not None and _stats.get("trace", False)
    try:
        res = run_bass_kernel_spmd(nc, in_maps, list(range(NCORES)), trace=trace)
    except ModuleNotFoundError:
        # NTFF profile hook unavailable in this environment
        res = run_bass_kernel_spmd(nc, in_maps, list(range(NCORES)), trace=False)
    if _stats is not None:
        _stats["exec_time_ns"] = res.exec_time_ns
        _stats["profile_json"] = res.profile_json
    return assemble(res.results, b_proj)

